# revision 25
# baseline (speedup 1.0000x reference)
"""Performer (FAVOR+, non-causal) attention block on 8 trn2 NeuronCores.

Sharding: sequence-parallel. Flattened tokens [B*S=16384, D] are split into 8
contiguous chunks of 2048; core c holds tokens of batch b=c//2. The KV
reduction over s is completed with a pairwise AllReduce (cores {2b, 2b+1}).

Per-core dataflow (all matmuls fp32r, PSUM fp32):
  pass 0: x chunk -> PE-transpose -> xT [d, tok] (feature-major)
          qT/kT = W^T @ xT (feature-major), v = xT^T @ Wv (token-major,
          augmented with a ones column per head for ksum)
          kf = exp(SCALE*(kT.rfT) - max_r) + EPS   (token-major)
          kv_psum[h] = v_aug_h^T @ kf_h  -> [65, 256] = [kv^T; ksum]
  AllReduce([65, 2048]) over core pairs.
  pass 2: qf = exp(SCALE*(rfT^T qT))  (feature-major, no max: cancels in
          num/den up to EPS*e^{max}~1e-4 relative)
          num+den = [kv;ksum]^T @ qf -> [65, tok]; attn = (num+eps)*recip(den)
          out = attn_fm^T @ Wo  (+bo added host-side; bo==0 in this problem)
"""

import numpy as np

import concourse.bass as bass
import concourse.mybir as mybir
import concourse.tile as tile
from concourse import bacc
from concourse.bass_utils import run_bass_kernel_spmd
from concourse.masks import make_identity

B, S, D = 4, 4096, 512
H, KD, R = 8, 64, 256
EPS = 1e-3
SCALE = 1.0 / float(np.sqrt(KD))
NC = 8
T = B * S // NC          # 2048 tokens per core
CH = 4                   # chunks per core
CT = T // CH             # 512 tokens per chunk
F32 = mybir.dt.float32
FR = mybir.dt.float32r
BF = mybir.dt.bfloat16
AX = mybir.AxisListType.X
ALU = mybir.AluOpType
ACTF = mybir.ActivationFunctionType


def _r(ap):
    return ap.bitcast(FR)


def build_kernel(tc, single=False):
    nc = tc.nc
    x_d = nc.dram_tensor("x", [T, D], F32, kind="ExternalInput").ap()
    wq_d = nc.dram_tensor("wq", [D, D], F32, kind="ExternalInput").ap()
    wk_d = nc.dram_tensor("wk", [D, D], F32, kind="ExternalInput").ap()
    wv_d = nc.dram_tensor("wv", [D, D], F32, kind="ExternalInput").ap()
    wo_d = nc.dram_tensor("wo", [D, D], F32, kind="ExternalInput").ap()
    rft_d = nc.dram_tensor("rft", [KD, R], F32, kind="ExternalInput").ap()
    out_d = nc.dram_tensor("out", [T, D], F32, kind="ExternalOutput").ap()

    cpool = tc.alloc_tile_pool(name="cpool", bufs=1)
    ppool = tc.alloc_tile_pool(name="ppool", bufs=1)
    sb = tc.alloc_tile_pool(name="sb", bufs=2)
    ps = tc.alloc_tile_pool(name="ps", bufs=2, space="PSUM")
    dram = tc.alloc_tile_pool(name="dram", bufs=1, space="DRAM")

    # ---- constants ----
    ident = cpool.tile([128, 128], F32)
    make_identity(nc, ident[:])
    ones0 = cpool.tile([128, 64], F32)
    nc.gpsimd.memset(ones0[:], 1.0)
    ones = cpool.tile([128, 64], F32)
    nc.scalar.copy(_r(ones[:]), ones0[:])
    # rfT duplicated on both partition halves (heads alternate 64-row halves)
    rft2 = cpool.tile([128, R], F32)
    nc.sync.dma_start(_r(rft2[0:KD, :]), _r(rft_d[:, :]))
    nc.sync.dma_start(_r(rft2[KD:128, :]), _r(rft_d[:, :]))
    # block-diagonal rfT for the kf projection (2 heads per matmul)
    rfbd0 = cpool.tile([128, 2 * R], F32)
    nc.gpsimd.memset(rfbd0[:], 0.0)
    nc.scalar.dma_start(rfbd0[0:KD, 0:R], rft_d[:, :])
    nc.scalar.dma_start(rfbd0[KD:128, R : 2 * R], rft_d[:, :])
    rfbd = cpool.tile([128, 2 * R], F32)
    nc.scalar.copy(_r(rfbd[:]), rfbd0[:])

    # ---- weights: w[k4-tile] = W[k4*128:(k4+1)*128, :] at cols [k4*512, +512)
    wsb = {}
    for name, d in (("wq", wq_d), ("wk", wk_d), ("wv", wv_d), ("wo", wo_d)):
        t = cpool.tile([128, 4 * D], F32, tag=f"w_{name}", name=f"w_{name}")
        nc.scalar.dma_start(
            _r(t.rearrange("p (a d) -> p a d", a=4)),
            _r(d.rearrange("(a p) d -> p a d", p=128)),
        )
        wsb[name] = t

    # persistent q^T (feature-major): rows = feat within m-tile, cols m*T + tok
    qT = ppool.tile([128, 4 * T], F32, tag="qT")

    # ---------------- pass 1 ----------------
    pskv = tc.alloc_tile_pool(name="pskv", bufs=2, space="PSUM")
    kvacc = ppool.tile([128, H * R], F32, tag="kvacc")
    nc.gpsimd.memset(kvacc[0:65, :], 0.0)

    for c in range(CH):
        # x -> xT (feature-major): PE transposes, 4 per PSUM bank, one DVE copy out
        xT = sb.tile([128, 4, CT], F32, tag="xT", bufs=2)
        xt = sb.tile([128, 4, D], F32, tag="xin", bufs=1)
        nc.sync.dma_start(
            xt[:, :, :],
            x_d[c * CT : (c + 1) * CT, :].rearrange("(a p) d -> p a d", p=128),
        )
        for t4 in range(4):
            pt = ps.tile([128, CT], F32, tag="ps")
            for k4 in range(4):
                nc.tensor.transpose(
                    pt[:, k4 * 128 : (k4 + 1) * 128], xt[:, t4, k4 * 128 : (k4 + 1) * 128], ident[:]
                )
            nc.vector.tensor_copy(
                _r(xT[:, :, t4 * 128 : (t4 + 1) * 128]),
                pt.rearrange("p (a t) -> p a t", a=4),
            )

        # q/k projections, feature-major out [feat, tok]
        kTc = sb.tile([128, 4, CT], F32, tag="kTc", bufs=1)
        for m in range(4):
            pq = ps.tile([128, CT], F32, tag="ps")
            pk = ps.tile([128, CT], F32, tag="ps")
            for k4 in range(4):
                lq = wsb["wq"][:, k4 * D + m * 128 : k4 * D + (m + 1) * 128]
                lk = wsb["wk"][:, k4 * D + m * 128 : k4 * D + (m + 1) * 128]
                nc.tensor.matmul(pq[:], _r(lq), _r(xT[:, k4, :]), start=(k4 == 0), stop=(k4 == 3))
                nc.tensor.matmul(pk[:], _r(lk), _r(xT[:, k4, :]), start=(k4 == 0), stop=(k4 == 3))
            nc.vector.tensor_copy(_r(qT[:, m * T + c * CT : m * T + (c + 1) * CT]), pq[:])
            nc.scalar.copy(_r(kTc[:, m, :]), pk[:])

        # v projection, token-major, augmented ones column per head
        vaug = sb.tile([128, 4, H * 65], BF, tag="vaug", bufs=1)
        nc.scalar.copy(vaug.rearrange("p a (h e) -> p a h e", e=65)[:, :, :, 64:65],
                       ones0[:, 0:32].rearrange("p (a h e) -> p a h e", a=4, h=H))
        for t4 in range(4):
            pv = ps.tile([128, CT], F32, tag="ps")
            for k4 in range(4):
                nc.tensor.matmul(
                    pv[:],
                    _r(xT[:, k4, t4 * 128 : (t4 + 1) * 128]),
                    _r(wsb["wv"][:, k4 * D : (k4 + 1) * D]),
                    start=(k4 == 0),
                    stop=(k4 == 3),
                )
            dst = vaug.rearrange("p a (h e) -> p a h e", e=65)[:, t4, :, 0:64]
            nc.vector.tensor_copy(dst, pv.rearrange("p (h e) -> p h e", e=64)[:, :, :])

        # kf features (token-major) + EPS, then kv accumulation
        for t4 in range(4):
            kfs = sb.tile([128, H * R], BF, tag="kfs")
            for hp in range(4):
                pf = ps.tile([128, CT], F32, tag="ps")
                nc.tensor.matmul(
                    pf[:], _r(kTc[:, hp, t4 * 128 : (t4 + 1) * 128]), _r(rfbd[:]),
                    start=True, stop=True,
                )
                nmx = sb.tile([128, 2], F32, tag="nmx")
                nc.vector.reduce_max(nmx[:], pf.rearrange("p (j r) -> p j r", j=2), axis=AX)
                nc.vector.tensor_scalar_mul(nmx[:], nmx[:], -SCALE)
                for j in (0, 1):
                    h = 2 * hp + j
                    nc.scalar.activation(
                        kfs[:, h * R : (h + 1) * R], pf[:, j * R : (j + 1) * R],
                        ACTF.Exp, bias=nmx[:, j : j + 1], scale=SCALE,
                    )
            nc.vector.tensor_scalar_add(kfs[:], kfs[:], EPS)
            for hq in range(2):
                pkv = pskv.tile([128, 2 * CT], F32, tag="pskv", name="pkv")
                for j in range(4):
                    h = 4 * hq + j
                    nc.tensor.matmul(
                        pkv[0:65, j * R : (j + 1) * R],
                        vaug[:, t4, h * 65 : (h + 1) * 65],
                        kfs[:, h * R : (h + 1) * R],
                        start=True, stop=True,
                    )
                nc.vector.scalar_tensor_tensor(
                    kvacc[0:65, hq * 4 * R : (hq + 1) * 4 * R], pkv[0:65, :], 1.0,
                    kvacc[0:65, hq * 4 * R : (hq + 1) * 4 * R], op0=ALU.mult, op1=ALU.add,
                )

    pskv.release()
    # ---- collective: pairwise AllReduce of [65, H*R] ----
    cin = dram.tile([65, H * R], F32, tag="cin")
    cout = dram.tile([65, H * R], F32, tag="cout")
    nc.gpsimd.dma_start(cin[:], kvacc[0:65, :])
    if single:
        nc.sync.dma_start(cout[:], cin[:])
    else:
        nc.gpsimd.collective_compute(
            "AllReduce", ALU.add,
            replica_groups=[[0, 1], [2, 3], [4, 5], [6, 7]],
            ins=[cin.opt()], outs=[cout.opt()],
        )
    kvar = ppool.tile([128, H * R], F32, tag="kvar")
    nc.gpsimd.dma_start(kvar[0:65, :], cout[:])

    # ---- post-AR: kvT_aug (lhsT for num+den) and eps vectors ----
    kvT = [ppool.tile([128, H * 65], BF, tag=f"kvT{rh}", name=f"kvT{rh}") for rh in (0, 1)]
    for h in range(H):
        for rh in (0, 1):
            pt = ps.tile([128, CT], F32, tag="ps")
            nc.tensor.transpose(
                pt[0:128, 0:65],
                kvar[0:65, h * R + rh * 128 : h * R + rh * 128 + 128],
                ident[0:65, 0:65],
            )
            nc.scalar.copy(kvT[rh][:, h * 65 : (h + 1) * 65], pt[0:128, 0:65])
    # em[:, h] = EPS * sum_r kv_ar[:, h, r]; row 64 gets extra +EPS (final den eps)
    em = ppool.tile([128, H], F32, tag="em")
    nc.vector.reduce_sum(em[0:65, :], kvar.rearrange("p (h r) -> p h r", r=R)[0:65, :, :], axis=AX)
    nc.vector.tensor_scalar_mul(em[0:65, :], em[0:65, :], EPS)
    nc.vector.tensor_scalar_add(em[64:65, :], em[64:65, :], EPS)

    # ---------------- pass 2 ----------------
    psn = tc.alloc_tile_pool(name="psn", bufs=2, space="PSUM")
    psq = tc.alloc_tile_pool(name="psq", bufs=2, space="PSUM")
    for c in range(CH):
        qfs = sb.tile([128, H * 2 * CT], BF, tag="qf", bufs=1)
        for h in range(H):
            p0 = (h % 2) * KD
            m = h // 2
            pq = psq.tile([128, 2 * CT], F32, tag="psq", name="pq")
            for rh in (0, 1):
                nc.tensor.matmul(
                    pq[:, rh * CT : (rh + 1) * CT],
                    _r(rft2[p0 : p0 + KD, rh * 128 : (rh + 1) * 128]),
                    _r(qT[p0 : p0 + KD, m * T + c * CT : m * T + (c + 1) * CT]),
                    start=True, stop=True,
                )
            nc.scalar.activation(qfs[:, h * 2 * CT : (h + 1) * 2 * CT], pq[:], ACTF.Exp, scale=SCALE)
        att = sb.tile([128, 4, CT], F32, tag="att", bufs=1)
        for h in range(H):
            pn = psn.tile([128, CT], F32, tag="pn")
            for rh in (0, 1):
                nc.tensor.matmul(
                    pn[0:65, :],
                    kvT[rh][:, h * 65 : (h + 1) * 65],
                    qfs[:, (h * 2 + rh) * CT : (h * 2 + rh + 1) * CT],
                    start=(rh == 0), stop=(rh == 1),
                )
            rc = sb.tile([128, CT], F32, tag="rc")
            nc.vector.tensor_scalar_add(_r(rc[64:65, :]), pn[64:65, :], em[64:65, h : h + 1])
            nc.vector.reciprocal(_r(rc[64:65, :]), _r(rc[64:65, :]))
            pb = ps.tile([128, CT], F32, tag="ps", name="pb")
            nc.tensor.matmul(pb[0:64, :], _r(ones[64:65, :]), _r(rc[64:65, :]), start=True, stop=True)
            tn = sb.tile([128, CT], F32, tag="tn")
            nc.scalar.activation(tn[0:64, :], pn[0:64, :], ACTF.Identity, bias=em[0:64, h : h + 1])
            if h % 2 == 0:
                nc.vector.scalar_tensor_tensor(
                    _r(att[0:64, h // 2, :]), tn[0:64, :], 1.0, pb[0:64, :],
                    op0=ALU.mult, op1=ALU.mult,
                )
            else:
                am = sb.tile([128, CT], F32, tag="am")
                nc.vector.scalar_tensor_tensor(
                    _r(am[0:64, :]), tn[0:64, :], 1.0, pb[0:64, :],
                    op0=ALU.mult, op1=ALU.mult,
                )
                nc.gpsimd.dma_start(_r(att[64:128, h // 2, :]), _r(am[0:64, :]))
        for t4 in range(4):
            po = ps.tile([128, CT], F32, tag="ps")
            for hp in range(4):
                nc.tensor.matmul(
                    po[:],
                    _r(att[:, hp, t4 * 128 : (t4 + 1) * 128]),
                    _r(wsb["wo"][:, hp * D : (hp + 1) * D]),
                    start=(hp == 0), stop=(hp == 3),
                )
            ob = sb.tile([128, D], F32, tag="ob")
            nc.scalar.copy(ob[:], po[:])
            r0 = (c * 4 + t4) * 128
            nc.sync.dma_start(out_d[r0 : r0 + 128, :], ob[:])

    psq.release()
    for p in (psn, dram, ps, sb, ppool, cpool):
        p.release()


_NC_CACHE = {}


def _get_nc():
    if "nc" not in _NC_CACHE:
        nc = bacc.Bacc("TRN2", target_bir_lowering=False, debug=False, num_devices=NC)
        with nc.allow_low_precision(reason="fp32r rounding of matmul inputs"):
            with tile.TileContext(nc) as tc:
                build_kernel(tc)
        nc.compile()
        _NC_CACHE["nc"] = nc
    return _NC_CACHE["nc"]


def kernel(x, Wq, Wk, Wv, rf, Wo, bo):
    nc = _get_nc()
    xf = np.ascontiguousarray(np.asarray(x, np.float32).reshape(B * S, D))
    rft = np.ascontiguousarray(np.asarray(rf, np.float32).T)
    ins = {
        "wq": np.ascontiguousarray(np.asarray(Wq, np.float32)),
        "wk": np.ascontiguousarray(np.asarray(Wk, np.float32)),
        "wv": np.ascontiguousarray(np.asarray(Wv, np.float32)),
        "wo": np.ascontiguousarray(np.asarray(Wo, np.float32)),
        "rft": rft,
    }
    in_maps = [{"x": xf[c * T : (c + 1) * T], **ins} for c in range(NC)]
    res = run_bass_kernel_spmd(nc, in_maps, core_ids=list(range(NC)))
    out = np.concatenate([res.results[c]["out"] for c in range(NC)], axis=0)
    out = out.reshape(B, S, D) + np.asarray(bo, np.float32)[None, None, :]
    return np.ascontiguousarray(out.astype(np.float32))
